# revision 1
# baseline (speedup 1.0000x reference)
"""LocalAttentionDraftLayer TRN2 Bass kernel.

Strategy: sequence-parallel over B*S across 8 cores (each core gets a
contiguous 1024-token chunk of one batch row, plus a 32-token halo of
preceding tokens, zero-padded at sequence start). Attention is strictly
local (window 32, causal), so no collectives are needed: the halo is
materialized host-side.

Everything on-chip is computed in "transposed land" ([feature, token]
layouts) so that every matmul contraction has its operand on partitions
without any transposes, except the attention probabilities P, which are
transposed on the PE (the classic flash-attention transpose).

Matmuls use dt.float32r (full-rate fp32 on the PE at N>=256, ~1e-4
scale-relative rounding); everything else is fp32.

Per core:
  QT[h,q]   = WqT.T @ xT        (scaled by 1/sqrt(H) on PSUM->SBUF copy)
  KT[h,k]   = WkT.T @ xT        (k padded to 1152 for N=256 score tiles)
  V[k,h]    = xT.T @ WvT        (9 chunks of 128 keys)
  per 128-query block b: scores[q, 256k] -> softmax -> P^T via PE
  per 256-query pair: attnT[h,q] += V.T @ P^T
  draftT    = WoT.T @ attnT + xT
  LN stats via ones-matmul partition reduction; rstd broadcast via K=1
  matmul; mean handled as a rank-1 K=1 correction matmul folded into the
  MLP; ln_w folded into W1 host-side, ln_b folded into the gelu bias.
  h1T       = gelu(W1wT.T @ (draftT*rstd) - w1sum*(mu*rstd) + bias1)
  outT      = W2T.T @ h1T + b2 + draftT
Host transposes outT back and stitches the 8 chunks.
"""

import sys

sys.path.insert(0, "/opt/trn_rl_repo")

from contextlib import ExitStack

import numpy as np

import concourse.bacc as bacc
import concourse.tile as tile
from concourse import mybir
from concourse.bass_utils import run_bass_kernel_spmd

B, S, H = 2, 4096, 1024
WIN = 32
N_CORES = 8
SL = S // 4            # 1024 tokens per core
XW = SL + WIN          # 1056 = halo + chunk
KW = SL + 128          # 1152 key-array width (pad so score tiles are N=256)
NB = SL // 128         # 8 query blocks
NP = NB // 2           # 4 query-block pairs

F32 = mybir.dt.float32
F32R = mybir.dt.float32r
AX = mybir.AxisListType.X
OP = mybir.AluOpType
AF = mybir.ActivationFunctionType

_CACHE = {}
DEBUG_TAPS = False


def _build():
    nc = bacc.Bacc("TRN2", target_bir_lowering=False, debug=False,
                   num_devices=N_CORES)

    def din(name, shape, dt=F32R):
        return nc.dram_tensor(name, shape, dt, kind="ExternalInput").ap()

    xT = din("xT", [H, XW])
    wq = din("wq", [H, H])
    wk = din("wk", [H, H])
    wv = din("wv", [H, H])
    wo = din("wo", [H, H])
    w1 = din("w1", [H, 512])
    w2 = din("w2", [512, H])
    cr_d = din("cr", [128, 1409])       # zeros|ones_c|ones_r(row0)|nw1s(row0)
    cf_d = din("cf", [128, 653], F32)   # m0|mR|ident|b1c|b2c|eps
    outT = nc.dram_tensor("outT", [H, SL], F32, kind="ExternalOutput").ap()
    taps = {}
    if DEBUG_TAPS:
        for nm, sh in [("t0_d", [128, 256]), ("t1_d", [128, 256]),
                       ("pn0_d", [128, 256]), ("pn1_d", [128, 256]),
                       ("ptg0_d", [128, 768]),
                       ("qt_d", [128, 8192]), ("kt_d", [128, 8 * KW]),
                       ("vt_d", [128, 9216]), ("at_d", [128, 8192]),
                       ("draft_d", [128, 8192]), ("drs_d", [128, 8192]),
                       ("h1_d", [128, 4096]), ("statr_d", [1, 3072])]:
            taps[nm] = nc.dram_tensor(nm, sh, F32, kind="ExternalOutput").ap()

    with tile.TileContext(nc) as tc, ExitStack() as ctx:
        sb = ctx.enter_context(tc.tile_pool(name="sb", bufs=1))
        sw = ctx.enter_context(tc.tile_pool(name="sw", bufs=3))
        sx = ctx.enter_context(tc.tile_pool(name="sx", bufs=2))
        ps = ctx.enter_context(tc.tile_pool(name="ps", bufs=3))
        ps4 = ctx.enter_context(tc.tile_pool(name="ps4", bufs=3, space="PSUM"))
        ps3 = ctx.enter_context(tc.tile_pool(name="ps3", bufs=4, space="PSUM"))
        ps1 = ctx.enter_context(tc.tile_pool(name="ps1", bufs=1, space="PSUM"))

        # ---- x^T first (critical path), then packed constants ----
        xt = sb.tile([128, 8 * XW], F32R, tag="xt")
        nc.sync.dma_start(xt[:, :].rearrange("p (c w) -> p c w", c=8),
                          xT.rearrange("(c p) w -> p c w", p=128))
        cr = sb.tile([128, 1409], F32R, tag="cr")
        cf = sb.tile([128, 653], F32, tag="cf")
        zero_sb = cr[:, 0:768]
        ones_c = cr[:, 768:769]
        ones_r = cr[0:1, 769:897]
        nw1s_sb = cr[0:1, 897:1409]
        m0_sb = cf[:, 0:256]
        mR_sb = cf[:, 256:512]
        ident_sb = cf[:, 512:640]
        b1c_sb = cf[:, 640:644]
        b2c_sb = cf[:, 644:652]
        eps_t = cf[0:1, 652:653]

        qt = sb.tile([128, 8 * 1024], F32R, tag="qt")
        kt = sb.tile([128, 8 * KW], F32R, tag="kt")
        vt = sb.tile([128, 9 * 1024], F32R, tag="vt")

        def load_quarter(w_dram, i, ncols=256, nkc=8):
            t = sw.tile([128, nkc * ncols], F32R, tag="w")
            nc.sync.dma_start(
                t[:, :].rearrange("p (c j) -> p c j", c=nkc),
                w_dram.rearrange("(c p) h -> p c h", p=128)
                [:, :, i * ncols:(i + 1) * ncols])
            return t

        # ---- Phase 1: QT = (Wq x^T) * 1/sqrt(H), layout [h-chunk][128, q] ----
        for i in range(4):
            wq_t = load_quarter(wq, i)
            for oc in (2 * i, 2 * i + 1):
                for qn in range(2):
                    pp = ps4.tile([128, 512], F32, tag="pp")
                    for kc in range(8):
                        nc.tensor.matmul(
                            pp[:, :],
                            wq_t[:, kc * 256 + (oc % 2) * 128:
                                 kc * 256 + (oc % 2) * 128 + 128],
                            xt[:, kc * XW + WIN + qn * 512:
                               kc * XW + WIN + (qn + 1) * 512],
                            start=(kc == 0), stop=(kc == 7))
                    nc.any.tensor_scalar_mul(
                        qt[:, oc * 1024 + qn * 512:oc * 1024 + (qn + 1) * 512],
                        pp[:, :], 1.0 / 32.0)

        # ---- Phase 2: KT, layout [h-chunk][128, 1152 keys] ----
        KNS = [(0, 384), (384, 384), (768, 288)]
        for i in range(4):
            wk_t = load_quarter(wk, i)
            for oc in (2 * i, 2 * i + 1):
                for (k0, kn) in KNS:
                    pp = ps4.tile([128, 512], F32, tag="pp")
                    for kc in range(8):
                        nc.tensor.matmul(
                            pp[:, 0:kn],
                            wk_t[:, kc * 256 + (oc % 2) * 128:
                                 kc * 256 + (oc % 2) * 128 + 128],
                            xt[:, kc * XW + k0:kc * XW + k0 + kn],
                            start=(kc == 0), stop=(kc == 7))
                    nc.any.tensor_copy(
                        kt[:, oc * KW + k0:oc * KW + k0 + kn], pp[:, 0:kn])

        # ---- Phase 3: V natural [key-chunk][128, h], 9 chunks ----
        for i in range(4):
            wv_t = load_quarter(wv, i)
            for vc in range(9):
                rows = 32 if vc == 8 else 128
                pp = ps4.tile([128, 512], F32, tag="pp")
                for kc in range(8):
                    nc.tensor.matmul(
                        pp[0:rows, 0:256],
                        xt[:, kc * XW + vc * 128:kc * XW + vc * 128 + rows],
                        wv_t[:, kc * 256:(kc + 1) * 256],
                        start=(kc == 0), stop=(kc == 7))
                nc.any.tensor_copy(
                    vt[0:rows, vc * 1024 + i * 256:vc * 1024 + (i + 1) * 256],
                    pp[0:rows, 0:256])

        if DEBUG_TAPS:
            nc.sync.dma_start(taps["qt_d"], qt[:, :].bitcast(F32))
            nc.sync.dma_start(taps["kt_d"], kt[:, :].bitcast(F32))
            nc.sync.dma_start(taps["vt_d"], vt[:, :].bitcast(F32))

        # consts arrive during the projection phases; pad keys before use
        nc.sync.dma_start(cr[:, :], cr_d)
        nc.sync.dma_start(cf[:, :], cf_d)
        for c in range(8):  # zero the key pad columns [1056, 1152)
            nc.vector.tensor_copy(kt[:, c * KW + XW:(c + 1) * KW],
                                  zero_sb[:, 0:KW - XW])

        # ---- Phase 4: local attention -> attnT [h-chunk][128, q] ----
        at = sb.tile([128, 8 * 1024], F32R, tag="xt")  # reuse xt slot
        for p in range(NP):
            ptg = sx.tile([128, 3 * 256], F32R, tag="ptg")
            nc.any.tensor_copy(ptg[:, :], zero_sb)
            for j in range(2):
                b = 2 * p + j
                sc = ps3.tile([128, 512], F32, tag="sc")
                for kc in range(8):
                    nc.tensor.matmul(
                        sc[:, 0:256],
                        qt[:, kc * 1024 + b * 128:kc * 1024 + (b + 1) * 128],
                        kt[:, kc * KW + b * 128:kc * KW + b * 128 + 256],
                        start=(kc == 0), stop=(kc == 7))
                t = sx.tile([128, 256], F32, tag="p")
                nc.vector.tensor_add(t[:, :], sc[:, 0:256],
                                     (m0_sb if b == 0 else mR_sb))
                nmax = sx.tile([128, 1], F32, tag="nm")
                nc.vector.reduce_max(nmax[:, :], t[:, :], axis=AX, negate=True)
                pexp = sx.tile([128, 256], F32, tag="pe")
                rsum = sx.tile([128, 1], F32, tag="rs")
                nc.scalar.activation(pexp[:, :], t[:, :], AF.Exp,
                                     bias=nmax[:, 0:1], scale=1.0,
                                     accum_out=rsum[:, 0:1])
                rcp = sx.tile([128, 1], F32, tag="rc")
                nc.vector.reciprocal(rcp[:, :], rsum[:, :])
                pn = sx.tile([128, 256], F32, tag="pn")
                nc.vector.tensor_scalar_mul(pn[:, :], pexp[:, :], rcp[:, 0:1])
                if DEBUG_TAPS and b < 2:
                    nc.sync.dma_start(taps[f"t{b}_d"], t[:, :])
                    nc.sync.dma_start(taps[f"pn{b}_d"], pn[:, :])
                # P^T pieces into the pair-group [288k x 256q] layout
                pt1 = ps1.tile([128, 512], F32, tag="pt", name="pt1")
                nc.tensor.transpose(pt1[:, 0:128], pn[:, 0:128], ident_sb)
                nc.any.tensor_copy(ptg[:, j * 384:j * 384 + 128],
                                   pt1[:, 0:128])
                pt2 = ps1.tile([128, 512], F32, tag="pt", name="pt2")
                nc.tensor.transpose(pt2[0:32, 0:128], pn[:, 128:160],
                                    ident_sb)
                nc.any.tensor_copy(ptg[0:32, 256 + j * 384:384 + j * 384],
                                   pt2[0:32, 0:128])
            if DEBUG_TAPS and p == 0:
                nc.sync.dma_start(taps["ptg0_d"], ptg[:, :].bitcast(F32))
            for hgr in range(4):
                # one accumulation region per PSUM bank: on HW, start=True
                # clears the whole bank, so groups must not share a bank
                atp = [ps3.tile([128, 256], F32, tag="sc", name=f"atp{hh}")
                       for hh in range(2)]
                for kc3 in range(3):
                    c = 2 * p + kc3
                    rows = 32 if c == 8 else 128
                    for hh in range(2):
                        hc = 2 * hgr + hh
                        nc.tensor.matmul(
                            atp[hh][:, :],
                            vt[0:rows, c * 1024 + hc * 128:
                               c * 1024 + (hc + 1) * 128],
                            ptg[0:rows, kc3 * 256:(kc3 + 1) * 256],
                            start=(kc3 == 0), stop=(kc3 == 2))
                for hh in range(2):
                    hc = 2 * hgr + hh
                    nc.any.tensor_copy(
                        at[:, hc * 1024 + p * 256:hc * 1024 + (p + 1) * 256],
                        atp[hh][:, :])

        if DEBUG_TAPS:
            nc.sync.dma_start(taps["at_d"], at[:, :].bitcast(F32))

        # ---- Phase 5+6: draftT = Wo attnT + xT; LN stats; drs = draft*rstd.
        # qn-outer so the qn=0 stats chain overlaps the qn=1 Wo matmuls.
        draft = sb.tile([128, 8 * 1024], F32R, tag="qt")  # reuse qt slot
        statr = sb.tile([1, 2048], F32R, tag="statr")
        drs = sb.tile([128, 8 * 1024], F32R, tag="kt")  # reuse kt slot
        for qn in range(2):
            s1 = ps3.tile([1, 512], F32, tag="sc", name=f"s1_{qn}")
            s2 = ps3.tile([1, 512], F32, tag="sc", name=f"s2_{qn}")
            for i in range(4):
                wo_t = load_quarter(wo, i)
                for oc in (2 * i, 2 * i + 1):
                    pp = ps4.tile([128, 512], F32, tag="pp")
                    for kc in range(8):
                        nc.tensor.matmul(
                            pp[:, :],
                            wo_t[:, kc * 256 + (oc % 2) * 128:
                                 kc * 256 + (oc % 2) * 128 + 128],
                            at[:, kc * 1024 + qn * 512:kc * 1024 + (qn + 1) * 512],
                            start=(kc == 0), stop=(kc == 7))
                    xr = sx.tile([128, 512], F32R, tag="xr")
                    nc.sync.dma_start(
                        xr[:, :],
                        xT[oc * 128:(oc + 1) * 128,
                           WIN + qn * 512:WIN + (qn + 1) * 512])
                    dsl = draft[:, oc * 1024 + qn * 512:oc * 1024 + (qn + 1) * 512]
                    nc.vector.tensor_add(dsl, pp[:, :], xr[:, :])
                    nc.tensor.matmul(s1[:, :], ones_c, dsl,
                                     start=(oc == 0), stop=(oc == 7))
                    sq = sx.tile([128, 512], F32R, tag="sq")
                    nc.scalar.square(sq[:, :], dsl)
                    nc.tensor.matmul(s2[:, :], ones_c, sq[:, :],
                                     start=(oc == 0), stop=(oc == 7))
            # stats chain for this qn (overlaps next qn's Wo matmuls)
            nc.vector.tensor_scalar_mul(s1[:, :], s1[:, :], 1.0 / H)
            # mu2 shares the rstd slice (consumed before rstd is written)
            mu2 = statr[0:1, qn * 512:(qn + 1) * 512]
            nc.scalar.square(mu2, s1[:, :])
            nc.vector.tensor_scalar_mul(s2[:, :], s2[:, :], 1.0 / H)
            nc.vector.tensor_sub(s2[:, :], s2[:, :], mu2)
            nc.scalar.activation(s2[:, :], s2[:, :], AF.Sqrt, bias=eps_t)
            rstd = statr[0:1, qn * 512:(qn + 1) * 512]
            with nc.allow_low_precision(reason="f32r is bit-identical to f32"):
                nc.vector.reciprocal(rstd, s2[:, :])
            nc.vector.tensor_mul(statr[0:1, 1024 + qn * 512:1024 + (qn + 1) * 512],
                                 s1[:, :], rstd)
            if qn == 0:
                rb = ps1.tile([128, 512], F32, tag="pt", name="rb")
                nc.tensor.matmul(rb[:, :], ones_r, rstd, start=True, stop=True)
                for oc in range(8):
                    sl = slice(oc * 1024, oc * 1024 + 512)
                    nc.vector.tensor_mul(drs[:, sl], draft[:, sl], rb[:, :])

        if DEBUG_TAPS:
            nc.sync.dma_start(taps["draft_d"], draft[:, :].bitcast(F32))
            nc.sync.dma_start(taps["drs_d"], drs[:, :].bitcast(F32))
            nc.sync.dma_start(taps["statr_d"], statr[:, :].bitcast(F32))

        # ---- Phase 7: h1T = gelu(W1w drs + mean-correction + bias1) ----
        # qn=0 groups first; qn=1's rstd broadcast + scaling is emitted after
        # them so the PE stream does not stall on the qn=1 LN stats chain.
        h1 = sb.tile([128, 4 * 1024], F32R, tag="vt")  # reuse vt slot

        def mlp1_group(w1_t, mc, qn):
            pp = ps4.tile([128, 512], F32, tag="pp", name="pp_m1")
            for kc in range(8):
                nc.tensor.matmul(
                    pp[:, :],
                    w1_t[:, kc * 256 + (mc % 2) * 128:
                         kc * 256 + (mc % 2) * 128 + 128],
                    drs[:, kc * 1024 + qn * 512:kc * 1024 + (qn + 1) * 512],
                    start=(kc == 0), stop=False)
            nc.tensor.matmul(
                pp[:, :],
                nw1s_sb[0:1, mc * 128:(mc + 1) * 128],
                statr[0:1, 1024 + qn * 512:1024 + (qn + 1) * 512],
                start=False, stop=True)
            nc.scalar.activation(
                h1[:, mc * 1024 + qn * 512:mc * 1024 + (qn + 1) * 512],
                pp[:, :], AF.Gelu, bias=b1c_sb[:, mc:mc + 1], scale=1.0)

        w1_ts = []
        for i in range(2):
            w1_t = load_quarter(w1, i)
            w1_ts.append(w1_t)
            for mc in (2 * i, 2 * i + 1):
                mlp1_group(w1_t, mc, 0)
        # deferred qn=1 scaling (hidden under the qn=0 MLP1 groups)
        rb1 = ps1.tile([128, 512], F32, tag="pt", name="rb1")
        nc.tensor.matmul(rb1[:, :], ones_r, statr[0:1, 512:1024],
                         start=True, stop=True)
        for oc in range(8):
            sl = slice(oc * 1024 + 512, oc * 1024 + 1024)
            nc.vector.tensor_mul(drs[:, sl], draft[:, sl], rb1[:, :])
        for i in range(2):
            for mc in (2 * i, 2 * i + 1):
                mlp1_group(w1_ts[i], mc, 1)

        if DEBUG_TAPS:
            nc.sync.dma_start(taps["h1_d"], h1[:, :].bitcast(F32))

        # ---- Phase 8: outT = W2 h1 + b2 + draftT ----
        for i in range(2):
            w2_t = sw.tile([128, 4 * 512], F32R, tag="w")
            nc.sync.dma_start(
                w2_t[:, :].rearrange("p (c j) -> p c j", c=4),
                w2.rearrange("(c p) h -> p c h", p=128)
                [:, :, i * 512:(i + 1) * 512])
            for oc in range(4 * i, 4 * i + 4):
                ot = sx.tile([128, 1024], F32, tag="ot")
                for qn in range(2):
                    pp = ps4.tile([128, 512], F32, tag="pp")
                    for mc in range(4):
                        nc.tensor.matmul(
                            pp[:, :],
                            w2_t[:, mc * 512 + (oc % 4) * 128:
                                 mc * 512 + (oc % 4) * 128 + 128],
                            h1[:, mc * 1024 + qn * 512:mc * 1024 + (qn + 1) * 512],
                            start=(mc == 0), stop=(mc == 3))
                    nc.vector.scalar_tensor_tensor(
                        ot[:, qn * 512:(qn + 1) * 512], pp[:, :],
                        b2c_sb[:, oc:oc + 1],
                        draft[:, oc * 1024 + qn * 512:oc * 1024 + (qn + 1) * 512],
                        op0=OP.add, op1=OP.add)
                nc.sync.dma_start(outT[oc * 128:(oc + 1) * 128, :], ot[:, :])

    nc.compile()
    return nc


def _get_nc():
    if "nc" not in _CACHE:
        _CACHE["nc"] = _build()
    return _CACHE["nc"]


def _masks():
    kk = np.arange(256)[None, :]
    p = np.arange(128)[:, None]
    band = (kk - p >= 1) & (kk - p <= WIN)
    mR = np.where(band, 0.0, -1e30).astype(np.float32)
    m_first = np.where(band & (kk >= WIN), 0.0, -1e30).astype(np.float32)
    return m_first, mR


def kernel(hidden_states, Wq, Wk, Wv, Wo, ln_w, ln_b, W1, b1, W2, b2):
    hs = np.ascontiguousarray(np.asarray(hidden_states, np.float32))
    Wq, Wk, Wv, Wo = (np.asarray(a, np.float32) for a in (Wq, Wk, Wv, Wo))
    ln_w, ln_b = np.asarray(ln_w, np.float32), np.asarray(ln_b, np.float32)
    W1, b1 = np.asarray(W1, np.float32), np.asarray(b1, np.float32)
    W2, b2 = np.asarray(W2, np.float32), np.asarray(b2, np.float32)

    nc = _get_nc()
    m_first, mR = _masks()
    w1T = np.ascontiguousarray(W1.T * ln_w[:, None])
    cr = np.zeros((128, 1409), np.float32)
    cr[:, 768] = 1.0
    cr[0, 769:897] = 1.0
    cr[0, 897:1409] = -w1T.sum(0)
    def cf_pack(m0):
        cf = np.zeros((128, 653), np.float32)
        cf[:, 0:256] = m0
        cf[:, 256:512] = mR
        cf[:, 512:640] = np.eye(128, dtype=np.float32)
        cf[:, 640:644] = (b1 + W1 @ ln_b).reshape(4, 128).T
        cf[:, 644:652] = b2.reshape(8, 128).T
        cf[0, 652] = 1e-5
        return cf
    cf_first, cf_rest = cf_pack(m_first), cf_pack(mR)
    shared = {
        "cr": cr,
        "wq": np.ascontiguousarray(Wq.T),
        "wk": np.ascontiguousarray(Wk.T),
        "wv": np.ascontiguousarray(Wv.T),
        "wo": np.ascontiguousarray(Wo.T),
        "w1": w1T,
        "w2": np.ascontiguousarray(W2.T),
    }
    in_maps = []
    for c in range(N_CORES):
        b, ch = divmod(c, 4)
        rows = hs[b, ch * SL:(ch + 1) * SL]
        halo = (np.zeros((WIN, H), np.float32) if ch == 0
                else hs[b, ch * SL - WIN:ch * SL])
        xT = np.ascontiguousarray(np.concatenate([halo, rows], 0).T)
        m = dict(shared)
        m["xT"] = xT
        m["cf"] = cf_first if ch == 0 else cf_rest
        in_maps.append(m)

    res = run_bass_kernel_spmd(nc, in_maps, list(range(N_CORES)))
    _CACHE["res"] = res
    out = np.empty((B, S, H), np.float32)
    for c in range(N_CORES):
        b, ch = divmod(c, 4)
        out[b, ch * SL:(ch + 1) * SL] = res.results[c]["outT"].T
    return out



# revision 9
# speedup vs baseline: 1.5784x; 1.5784x over previous
"""LocalAttentionDraftLayer TRN2 Bass kernel (v2, bf16 + fused weights).

Sharding: sequence-parallel over B*S across 8 cores (each core owns a
contiguous 1024-token chunk of one batch row plus a 32-token halo of
preceding tokens, zero-padded at sequence start). Attention is strictly
local (window 32, causal), so the halo is materialized host-side and no
collectives are needed.

Key algebraic restructurings vs the straightforward layer (all host-side
weight folds, validated to rel err 4.6e-3 in numpy with bf16 operands):

  scores = (x Wq^T)(x Wk^T)^T / sqrt(H) = (x A) x^T  with A = Wq^T Wk / sqrt(H)
    -> the K projection disappears; keys are just x^T (already resident).
  attn_out = P (x Wv^T) Wo^T = (P x) Wvo^T          with Wvo = Wo Wv
    -> the V projection disappears; PV runs directly on x, and the old
       Wo-sized GEMM becomes the only post-attention projection.

Everything on-chip lives in "transposed land" ([feature, token]); all
matmul operands are bf16 (cost model: 1 cycle/row at any N, PSUM f32),
roughly 247K PE cycles/core vs 411K for the fp32r baseline.

Per core:
  qt[h,q]  = A^T x^T                      (q~ = xA, includes 1/sqrt(H))
  per 128-query block b (keys = 160-wide slice of x^T starting at b*128):
    sc = mask_preload + q~ . x            (PSUM-preloaded additive mask,
                                           accumulate with start=False)
    softmax without max-subtraction (scores are O(5); exp is safe):
    pexp = exp(sc) with accum -> rsum; rcp = 1/rsum (DVE);
    pn = pexp * rcp -> bf16 (Act copy-with-scale)
    P^T via two PE transposes (128 + 32 key rows) -> ptg (SBUF)
  y[h,q]   = x_nat^T P^T  per (h-chunk, 512-query half) PSUM group
  attnT    = Wvo y; draftT = attnT + x^T; LN stats via ones-matmul;
  rstd broadcast via K=1 matmul; mean correction as rank-1 matmul folded
  into MLP1 (ln_w folded into W1, ln_b folded into the gelu bias).
  h1 = gelu(W1w drs + nw1s*(mu*rstd) + b1c); outT = W2 h1 + b2 + draftT
Host transposes outT back and stitches the 8 chunks.
"""

import sys

sys.path.insert(0, "/opt/trn_rl_repo")

from contextlib import ExitStack

import numpy as np
import ml_dtypes

import concourse.bacc as bacc
import concourse.tile as tile
from concourse import mybir
from concourse.bass_utils import run_bass_kernel_spmd

B, S, H = 2, 4096, 1024
WIN = 32
N_CORES = 8
SL = S // 4            # 1024 tokens per core
XW = SL + WIN          # 1056 = halo + chunk
NB = SL // 128         # 8 query blocks
KWIN = 160             # keys per query block (128-aligned window cover)

F32 = mybir.dt.float32
BF16 = mybir.dt.bfloat16
OP = mybir.AluOpType
AF = mybir.ActivationFunctionType

_CACHE = {}
DEBUG_TAPS = False


def _build():
    nc = bacc.Bacc("TRN2", target_bir_lowering=False, debug=False,
                   num_devices=N_CORES)

    def din(name, shape, dt=BF16):
        return nc.dram_tensor(name, shape, dt, kind="ExternalInput").ap()

    xT_d = din("xT", [H, XW])
    xn_d = din("xn", [9 * 128, H])
    a_d = din("a", [H, H])
    wvo_d = din("wvo", [H, H])
    w1_d = din("w1", [H, 512])
    w2_d = din("w2", [512, H])
    cb_d = din("cb", [128, 769])          # ident|ones_c|ones_r(row0)|nw1s(row0)
    cf_d = din("cf", [128, 333], F32)     # m0|mR|b1c|b2c|eps
    outT = nc.dram_tensor("outT", [H, SL], BF16, kind="ExternalOutput").ap()
    taps = {}
    if DEBUG_TAPS:
        for nm, sh, dt in [("qt_d", [128, 8192], BF16),
                           ("pn0_d", [128, 160], BF16),
                           ("ptg_d", [128, 2048], BF16),
                           ("ysb_d", [128, 8192], BF16),
                           ("draft_d", [128, 8192], BF16),
                           ("drs_d", [128, 8192], BF16),
                           ("h1_d", [128, 4096], BF16),
                           ("stat_d", [1, 1024], F32),
                           ("statb_d", [1, 2048], BF16)]:
            taps[nm] = nc.dram_tensor(nm, sh, dt, kind="ExternalOutput").ap()

    with tile.TileContext(nc) as tc, ExitStack() as ctx:
        sb = ctx.enter_context(tc.tile_pool(name="sb", bufs=1))
        sx = ctx.enter_context(tc.tile_pool(name="sx", bufs=2))
        pp = ctx.enter_context(tc.tile_pool(name="pp", bufs=2, space="PSUM"))
        sp = ctx.enter_context(tc.tile_pool(name="sp", bufs=6, space="PSUM"))

        # ---- SBUF tiles ----
        xt = sb.tile([128, 8 * XW], BF16, tag="xt")
        xn = sb.tile([128, 9 * 1024], BF16, tag="xn")
        qt = sb.tile([128, 8 * 1024], BF16, tag="qt")
        ptg = sb.tile([128, 8 * 256], BF16, tag="ptg")
        ysb = sb.tile([128, 8 * 1024], BF16, tag="ysb")
        draft = sb.tile([128, 8 * 1024], BF16, tag="draft")
        drs = sb.tile([128, 8 * 1024], BF16, tag="drs")
        h1 = sb.tile([128, 4 * 1024], BF16, tag="h1")
        statr = sb.tile([1, 1024], F32, tag="statr")
        statb = sb.tile([1, 2048], BF16, tag="statb")
        cb = sb.tile([128, 769], BF16, tag="cb")
        cf = sb.tile([128, 333], F32, tag="cf")
        aq = [sb.tile([128, 2048], BF16, tag=f"a{i}", name=f"aq{i}")
              for i in range(4)]
        wvq = [sb.tile([128, 2048], BF16, tag=f"wv{i}", name=f"wvq{i}")
               for i in range(4)]
        w1q = [sb.tile([128, 2048], BF16, tag=f"w1{i}", name=f"w1q{i}")
               for i in range(2)]
        w2h = [sb.tile([128, 2048], BF16, tag=f"w2{i}", name=f"w2h{i}")
               for i in range(2)]

        identb = cb[:, 0:128]
        ones_c = cb[:, 128:129]
        ones_r = cb[0:1, 129:257]
        nw1s = cb[0:1, 257:769]
        m0_sb = cf[:, 0:KWIN]
        mR_sb = cf[:, KWIN:2 * KWIN]
        b1c = cf[:, 320:324]
        b2c = cf[:, 324:332]
        eps_t = cf[0:1, 332:333]

        # ---- DMAs, in deadline order ----
        nc.sync.dma_start(cb, cb_d)
        nc.sync.dma_start(cf, cf_d)

        def dma_quarter(dst, src, i, ncols, nchunks):
            nc.sync.dma_start(
                dst[:, :].rearrange("p (c j) -> p c j", c=nchunks),
                src.rearrange("(c p) h -> p c h", p=128)
                [:, :, i * ncols:(i + 1) * ncols])

        xtv = xt[:, :].rearrange("p (c w) -> p c w", c=8)
        xTv = xT_d.rearrange("(c p) w -> p c w", p=128)
        dma_quarter(aq[0], a_d, 0, 256, 8)
        nc.sync.dma_start(xtv[:, :, WIN:WIN + 512], xTv[:, :, WIN:WIN + 512])
        for i in range(1, 4):
            dma_quarter(aq[i], a_d, i, 256, 8)
        nc.sync.dma_start(xtv[:, :, WIN + 512:XW], xTv[:, :, WIN + 512:XW])
        nc.sync.dma_start(xtv[:, :, 0:WIN], xTv[:, :, 0:WIN])
        nc.sync.dma_start(xn[:, :].rearrange("p (c h) -> p c h", c=9),
                          xn_d.rearrange("(c p) h -> p c h", p=128))
        for i in range(4):
            dma_quarter(wvq[i], wvo_d, i, 256, 8)
        for i in range(2):
            dma_quarter(w1q[i], w1_d, i, 256, 8)
        for i in range(2):
            dma_quarter(w2h[i], w2_d, i, 512, 4)

        # ---- Phase 1: qt = (x A)^T, layout [h-chunk][128, q] ----
        for qn in range(2):
            for i in range(4):
                for oc in (2 * i, 2 * i + 1):
                    ppt = pp.tile([128, 512], F32, tag="pp",
                                  name=f"q_{oc}_{qn}")
                    for kc in range(8):
                        nc.tensor.matmul(
                            ppt[:, :],
                            aq[i][:, kc * 256 + (oc % 2) * 128:
                                  kc * 256 + (oc % 2) * 128 + 128],
                            xt[:, kc * XW + WIN + qn * 512:
                               kc * XW + WIN + (qn + 1) * 512],
                            start=(kc == 0), stop=(kc == 7))
                    nc.any.tensor_copy(
                        qt[:, oc * 1024 + qn * 512:oc * 1024 + (qn + 1) * 512],
                        ppt[:, :])
        if DEBUG_TAPS:
            nc.sync.dma_start(taps["qt_d"], qt[:, :])

        # ---- Phase 2: local attention ----
        # Per 128-query block b, keys live at x^T cols [b*128, b*128+160);
        # the additive band mask (preloaded into PSUM) zeroes the rest.
        def sc_block(b):
            sct = sp.tile([128, KWIN], F32, tag="sp", name=f"sc{b}")
            for kc in range(8):
                nc.tensor.matmul(
                    sct[:, :],
                    qt[:, kc * 1024 + b * 128:kc * 1024 + (b + 1) * 128],
                    xt[:, kc * XW + b * 128:kc * XW + b * 128 + KWIN],
                    start=(kc == 0), stop=(kc == 7))
            return sct

        def softmax_block(b, sct):
            msl = m0_sb if b == 0 else mR_sb
            tt = sx.tile([128, KWIN], F32, tag="tt", name=f"tt{b}")
            nc.vector.tensor_add(tt[:, :], sct[:, :], msl)
            pexp = sx.tile([128, KWIN], F32, tag="pe", name=f"pe{b}")
            rsum = sx.tile([128, 1], F32, tag="rs", name=f"rs{b}")
            nc.scalar.activation(pexp[:, :], tt[:, :], AF.Exp,
                                 accum_out=rsum[:, 0:1])
            rcp = sx.tile([128, 1], F32, tag="rc", name=f"rc{b}")
            nc.vector.reciprocal(rcp[:, :], rsum[:, :])
            pn = sx.tile([128, KWIN], BF16, tag="pn", name=f"pn{b}")
            nc.scalar.activation(pn[:, :], pexp[:, :], AF.Copy,
                                 scale=rcp[:, 0:1])
            return pn

        def pt_block(b, pn):
            ptA = sp.tile([128, 128], BF16, tag="sp", name=f"ptA{b}")
            nc.tensor.transpose(ptA[:, 0:128], pn[:, 0:128], identb)
            nc.any.tensor_copy(ptg[:, b * 256:b * 256 + 128], ptA[:, 0:128])
            ptB = sp.tile([128, 128], BF16, tag="sp", name=f"ptB{b}")
            nc.tensor.transpose(ptB[0:32, 0:128], pn[:, 128:KWIN], identb)
            nc.any.tensor_copy(ptg[0:32, b * 256 + 128:b * 256 + 256],
                               ptB[0:32, 0:128])

        def y_group(oc, half):
            ypt = pp.tile([128, 512], F32, tag="pp", name=f"y{oc}_{half}")
            for j, bb in enumerate(range(half * 4, half * 4 + 4)):
                nc.tensor.matmul(
                    ypt[:, j * 128:(j + 1) * 128],
                    xn[:, bb * 1024 + oc * 128:bb * 1024 + oc * 128 + 128],
                    ptg[:, bb * 256:bb * 256 + 128],
                    start=(j == 0), stop=False)
            for j, bb in enumerate(range(half * 4, half * 4 + 4)):
                nc.tensor.matmul(
                    ypt[:, j * 128:(j + 1) * 128],
                    xn[0:32, (bb + 1) * 1024 + oc * 128:
                       (bb + 1) * 1024 + oc * 128 + 128],
                    ptg[0:32, bb * 256 + 128:bb * 256 + 256],
                    start=False, stop=(j == 3))
            nc.any.tensor_copy(
                ysb[:, oc * 1024 + half * 512:oc * 1024 + (half + 1) * 512],
                ypt[:, :])

        scts = {b: sc_block(b) for b in range(4)}
        for b in range(NB):
            pn = softmax_block(b, scts[b])
            if DEBUG_TAPS and b == 0:
                nc.sync.dma_start(taps["pn0_d"], pn[:, :])
            if b >= 4:
                y_group(2 * (b - 4), 0)
                y_group(2 * (b - 4) + 1, 0)
            pt_block(b, pn)
            if b + 4 < NB:
                scts[b + 4] = sc_block(b + 4)
        for oc in range(8):
            y_group(oc, 1)

        if DEBUG_TAPS:
            nc.sync.dma_start(taps["ptg_d"], ptg[:, :])
            nc.sync.dma_start(taps["ysb_d"], ysb[:, :])

        # ---- Phase 3: draftT = Wvo y + x^T; LN stats; drs = draft*rstd ----
        # qn-outer; stats matmuls trail their Wvo group by 2+ groups so the
        # PE never waits on the DVE add / Act square chain, and the qn=0
        # stats chain + rstd broadcast hide under the qn=1 Wvo matmuls.
        s_tiles = {}
        dsls = {}
        sqs = {}

        def wvo_group(qn, i, oc):
            ppt = pp.tile([128, 512], F32, tag="pp", name=f"wv_{oc}_{qn}")
            for kc in range(8):
                nc.tensor.matmul(
                    ppt[:, :],
                    wvq[i][:, kc * 256 + (oc % 2) * 128:
                           kc * 256 + (oc % 2) * 128 + 128],
                    ysb[:, kc * 1024 + qn * 512:kc * 1024 + (qn + 1) * 512],
                    start=(kc == 0), stop=(kc == 7))
            dsl = draft[:, oc * 1024 + qn * 512:oc * 1024 + (qn + 1) * 512]
            nc.vector.tensor_add(
                dsl, ppt[:, :],
                xt[:, oc * XW + WIN + qn * 512:oc * XW + WIN + (qn + 1) * 512])
            sq = sx.tile([128, 512], BF16, tag="sq", name=f"sq_{oc}_{qn}",
                         bufs=4)
            nc.scalar.square(sq[:, :], dsl)
            dsls[(qn, oc)] = dsl
            sqs[(qn, oc)] = sq

        def stat_mms(qn, oc):
            if oc == 0:
                s_tiles[qn] = (
                    sp.tile([1, 512], F32, tag="sp", name=f"s1_{qn}"),
                    sp.tile([1, 512], F32, tag="sp", name=f"s2_{qn}"))
            s1, s2 = s_tiles[qn]
            nc.tensor.matmul(s1[:, :], ones_c, dsls[(qn, oc)],
                             start=(oc == 0), stop=(oc == 7))
            nc.tensor.matmul(s2[:, :], ones_c, sqs[(qn, oc)][:, :],
                             start=(oc == 0), stop=(oc == 7))

        def stats_chain(qn):
            s1, s2 = s_tiles[qn]
            nc.vector.tensor_scalar_mul(s1[:, :], s1[:, :], 1.0 / H)
            mu2 = statr[0:1, qn * 512:(qn + 1) * 512]
            nc.scalar.square(mu2, s1[:, :])
            nc.vector.tensor_scalar_mul(s2[:, :], s2[:, :], 1.0 / H)
            nc.vector.tensor_sub(s2[:, :], s2[:, :], mu2)
            nc.scalar.activation(s2[:, :], s2[:, :], AF.Sqrt, bias=eps_t)
            rstd = statr[0:1, qn * 512:(qn + 1) * 512]
            nc.vector.reciprocal(rstd, s2[:, :])
            with nc.allow_low_precision(reason="bf16 operand copies of stats"):
                nc.vector.tensor_mul(
                    statb[0:1, 1024 + qn * 512:1024 + (qn + 1) * 512],
                    s1[:, :], rstd)
                nc.vector.tensor_copy(statb[0:1, qn * 512:(qn + 1) * 512],
                                      rstd)

        def rb_broadcast(qn):
            rb = pp.tile([128, 512], F32, tag="pp", name=f"rb{qn}")
            nc.tensor.matmul(rb[:, :], ones_r,
                             statb[0:1, qn * 512:(qn + 1) * 512],
                             start=True, stop=True)
            rbs = sx.tile([128, 512], F32, tag="rb", name=f"rbs{qn}")
            nc.any.tensor_copy(rbs[:, :], rb[:, :])
            return rbs

        def drs_muls(qn, rbs):
            for oc in range(8):
                sl = slice(oc * 1024 + qn * 512, oc * 1024 + qn * 512 + 512)
                nc.any.tensor_mul(drs[:, sl], draft[:, sl], rbs[:, :])

        for i in range(4):
            for oc in (2 * i, 2 * i + 1):
                wvo_group(0, i, oc)
                if oc >= 2:
                    stat_mms(0, oc - 2)

        def mlp1_group(i, mc, qn):
            ppt = pp.tile([128, 512], F32, tag="pp", name=f"m1_{mc}_{qn}")
            for kc in range(8):
                nc.tensor.matmul(
                    ppt[:, :],
                    w1q[i][:, kc * 256 + (mc % 2) * 128:
                           kc * 256 + (mc % 2) * 128 + 128],
                    drs[:, kc * 1024 + qn * 512:kc * 1024 + (qn + 1) * 512],
                    start=(kc == 0), stop=False)
            nc.tensor.matmul(
                ppt[:, :],
                nw1s[0:1, mc * 128:(mc + 1) * 128],
                statb[0:1, 1024 + qn * 512:1024 + (qn + 1) * 512],
                start=False, stop=True)
            nc.scalar.activation(
                h1[:, mc * 1024 + qn * 512:mc * 1024 + (qn + 1) * 512],
                ppt[:, :], AF.Gelu, bias=b1c[:, mc:mc + 1], scale=1.0)

        # qn=1 Wvo groups, with the qn=0 stats tail / broadcast interleaved
        wvo_group(1, 0, 0)
        stat_mms(0, 6)
        wvo_group(1, 0, 1)
        stat_mms(0, 7)
        stats_chain(0)
        wvo_group(1, 1, 2)
        rbs0 = rb_broadcast(0)
        drs_muls(0, rbs0)
        wvo_group(1, 1, 3)
        stat_mms(1, 0)
        wvo_group(1, 2, 4)
        stat_mms(1, 1)
        wvo_group(1, 2, 5)
        stat_mms(1, 2)
        wvo_group(1, 3, 6)
        stat_mms(1, 3)
        wvo_group(1, 3, 7)
        stat_mms(1, 4)

        if DEBUG_TAPS:
            nc.sync.dma_start(taps["draft_d"], draft[:, :])

        # MLP1 qn=0 groups, with the qn=1 stats tail / broadcast interleaved
        mlp1_group(0, 0, 0)
        stat_mms(1, 5)
        mlp1_group(0, 1, 0)
        stat_mms(1, 6)
        mlp1_group(1, 2, 0)
        stat_mms(1, 7)
        stats_chain(1)
        mlp1_group(1, 3, 0)
        rbs1 = rb_broadcast(1)
        drs_muls(1, rbs1)
        for i in range(2):
            for mc in (2 * i, 2 * i + 1):
                mlp1_group(i, mc, 1)

        if DEBUG_TAPS:
            nc.sync.dma_start(taps["stat_d"], statr[:, :])
            nc.sync.dma_start(taps["statb_d"], statb[:, :])
            nc.sync.dma_start(taps["drs_d"], drs[:, :])
            nc.sync.dma_start(taps["h1_d"], h1[:, :])

        # ---- Phase 5: outT = W2 h1 + b2 + draftT ----
        for i in range(2):
            for oc in range(4 * i, 4 * i + 4):
                ot = sx.tile([128, 1024], BF16, tag="ot")
                for qn in range(2):
                    ppt = pp.tile([128, 512], F32, tag="pp",
                                  name=f"m2_{oc}_{qn}")
                    for mc in range(4):
                        nc.tensor.matmul(
                            ppt[:, :],
                            w2h[i][:, mc * 512 + (oc % 4) * 128:
                                   mc * 512 + (oc % 4) * 128 + 128],
                            h1[:, mc * 1024 + qn * 512:
                               mc * 1024 + (qn + 1) * 512],
                            start=(mc == 0), stop=(mc == 3))
                    nc.vector.scalar_tensor_tensor(
                        ot[:, qn * 512:(qn + 1) * 512], ppt[:, :],
                        b2c[:, oc:oc + 1],
                        draft[:, oc * 1024 + qn * 512:
                              oc * 1024 + (qn + 1) * 512],
                        op0=OP.add, op1=OP.add)
                nc.sync.dma_start(outT[oc * 128:(oc + 1) * 128, :], ot[:, :])

    nc.compile()
    return nc


def _get_nc():
    if "nc" not in _CACHE:
        _CACHE["nc"] = _build()
    return _CACHE["nc"]


def _masks():
    kk = np.arange(KWIN)[None, :]
    p = np.arange(128)[:, None]
    band = (kk - p >= 1) & (kk - p <= WIN)
    mR = np.where(band, 0.0, -1e30).astype(np.float32)
    m_first = np.where(band & (kk >= WIN), 0.0, -1e30).astype(np.float32)
    return m_first, mR


def kernel(hidden_states, Wq, Wk, Wv, Wo, ln_w, ln_b, W1, b1, W2, b2):
    bf16 = ml_dtypes.bfloat16
    hs = np.ascontiguousarray(np.asarray(hidden_states, np.float32))
    Wq, Wk, Wv, Wo = (np.asarray(a, np.float32) for a in (Wq, Wk, Wv, Wo))
    ln_w, ln_b = np.asarray(ln_w, np.float32), np.asarray(ln_b, np.float32)
    W1, b1 = np.asarray(W1, np.float32), np.asarray(b1, np.float32)
    W2, b2 = np.asarray(W2, np.float32), np.asarray(b2, np.float32)

    nc = _get_nc()
    m_first, mR = _masks()
    A = (Wq.T @ Wk) / np.sqrt(H)
    Wvo = Wo @ Wv
    w1T = W1.T * ln_w[:, None]
    cbm = np.zeros((128, 769), np.float32)
    cbm[:, 0:128] = np.eye(128, dtype=np.float32)
    cbm[:, 128] = 1.0
    cbm[0, 129:257] = 1.0
    cbm[0, 257:769] = -w1T.sum(0)

    def cf_pack(m0):
        cfm = np.zeros((128, 333), np.float32)
        cfm[:, 0:KWIN] = m0
        cfm[:, KWIN:2 * KWIN] = mR
        cfm[:, 320:324] = (b1 + W1 @ ln_b).reshape(4, 128).T
        cfm[:, 324:332] = b2.reshape(8, 128).T
        cfm[0, 332] = 1e-5
        return cfm
    cf_first, cf_rest = cf_pack(m_first), cf_pack(mR)
    shared = {
        "cb": cbm.astype(bf16),
        "a": np.ascontiguousarray(A).astype(bf16),
        "wvo": np.ascontiguousarray(Wvo.T).astype(bf16),
        "w1": np.ascontiguousarray(w1T).astype(bf16),
        "w2": np.ascontiguousarray(W2.T).astype(bf16),
    }
    in_maps = []
    for c in range(N_CORES):
        b, ch = divmod(c, 4)
        rows = hs[b, ch * SL:(ch + 1) * SL]
        halo = (np.zeros((WIN, H), np.float32) if ch == 0
                else hs[b, ch * SL - WIN:ch * SL])
        xfull = np.concatenate([halo, rows], 0)          # [1056, H]
        xn = np.zeros((9 * 128, H), np.float32)
        xn[0:XW] = xfull
        m = dict(shared)
        m["xT"] = np.ascontiguousarray(xfull.T).astype(bf16)
        m["xn"] = xn.astype(bf16)
        m["cf"] = cf_first if ch == 0 else cf_rest
        in_maps.append(m)

    res = run_bass_kernel_spmd(nc, in_maps, list(range(N_CORES)))
    _CACHE["res"] = res
    out = np.empty((B, S, H), np.float32)
    for c in range(N_CORES):
        b, ch = divmod(c, 4)
        out[b, ch * SL:(ch + 1) * SL] = \
            res.results[c]["outT"].T.astype(np.float32)
    return out


# revision 16
# speedup vs baseline: 1.6147x; 1.0230x over previous
"""LocalAttentionDraftLayer TRN2 Bass kernel (v2, bf16 + fused weights).

Sharding: sequence-parallel over B*S across 8 cores (each core owns a
contiguous 1024-token chunk of one batch row plus a 32-token halo of
preceding tokens, zero-padded at sequence start). Attention is strictly
local (window 32, causal), so the halo is materialized host-side and no
collectives are needed.

Key algebraic restructurings vs the straightforward layer (all host-side
weight folds, validated to rel err 4.6e-3 in numpy with bf16 operands):

  scores = (x Wq^T)(x Wk^T)^T / sqrt(H) = (x A) x^T  with A = Wq^T Wk / sqrt(H)
    -> the K projection disappears; keys are just x^T (already resident).
  attn_out = P (x Wv^T) Wo^T = (P x) Wvo^T          with Wvo = Wo Wv
    -> the V projection disappears; PV runs directly on x, and the old
       Wo-sized GEMM becomes the only post-attention projection.

Everything on-chip lives in "transposed land" ([feature, token]); all
matmul operands are bf16 (cost model: 1 cycle/row at any N, PSUM f32),
roughly 247K PE cycles/core vs 411K for the fp32r baseline.

Per core:
  qt[h,q]  = A^T x^T                      (q~ = xA, includes 1/sqrt(H))
  per 128-query block b (keys = 160-wide slice of x^T starting at b*128):
    sc = mask_preload + q~ . x            (PSUM-preloaded additive mask,
                                           accumulate with start=False)
    softmax without max-subtraction (scores are O(5); exp is safe):
    pexp = exp(sc) with accum -> rsum; rcp = 1/rsum (DVE);
    pn = pexp * rcp -> bf16 (Act copy-with-scale)
    P^T via two PE transposes (128 + 32 key rows) -> ptg (SBUF)
  y[h,q]   = x_nat^T P^T  per (h-chunk, 512-query half) PSUM group
  attnT    = Wvo y; draftT = attnT + x^T; LN stats via ones-matmul;
  rstd broadcast via K=1 matmul; mean correction as rank-1 matmul folded
  into MLP1 (ln_w folded into W1, ln_b folded into the gelu bias).
  h1 = gelu(W1w drs + nw1s*(mu*rstd) + b1c); outT = W2 h1 + b2 + draftT
Host transposes outT back and stitches the 8 chunks.
"""

import sys

sys.path.insert(0, "/opt/trn_rl_repo")

from contextlib import ExitStack

import numpy as np
import ml_dtypes

import concourse.bacc as bacc
import concourse.tile as tile
from concourse import mybir
from concourse.bass_utils import run_bass_kernel_spmd

B, S, H = 2, 4096, 1024
WIN = 32
N_CORES = 8
SL = S // 4            # 1024 tokens per core
XW = SL + WIN          # 1056 = halo + chunk
NB = SL // 128         # 8 query blocks
KWIN = 160             # keys per query block (128-aligned window cover)

F32 = mybir.dt.float32
BF16 = mybir.dt.bfloat16
OP = mybir.AluOpType
AF = mybir.ActivationFunctionType

_CACHE = {}
DEBUG_TAPS = False


def _build():
    nc = bacc.Bacc("TRN2", target_bir_lowering=False, debug=False,
                   num_devices=N_CORES)

    def din(name, shape, dt=BF16):
        return nc.dram_tensor(name, shape, dt, kind="ExternalInput").ap()

    xT_d = din("xT", [H, XW])
    xn_d = din("xn", [9 * 128, H])
    a_d = din("a", [H, H])
    wvo_d = din("wvo", [H, H])
    w1_d = din("w1", [H, 512])
    w2_d = din("w2", [512, H])
    cb_d = din("cb", [128, 769])          # ident|ones_c|ones_r(row0)|nw1s(row0)
    cf_d = din("cf", [128, 333], F32)     # m0|mR|b1c|b2c|eps
    outT = nc.dram_tensor("outT", [H, SL], BF16, kind="ExternalOutput").ap()
    taps = {}
    if DEBUG_TAPS:
        for nm, sh, dt in [("qt_d", [128, 8192], BF16),
                           ("pn0_d", [128, 160], BF16),
                           ("ptg_d", [128, 2048], BF16),
                           ("ysb_d", [128, 8192], BF16),
                           ("draft_d", [128, 8192], BF16),
                           ("drs_d", [128, 8192], BF16),
                           ("h1_d", [128, 4096], BF16),
                           ("stat_d", [1, 1024], F32),
                           ("statb_d", [1, 2048], BF16)]:
            taps[nm] = nc.dram_tensor(nm, sh, dt, kind="ExternalOutput").ap()

    with tile.TileContext(nc) as tc, ExitStack() as ctx:
        sb = ctx.enter_context(tc.tile_pool(name="sb", bufs=1))
        sx = ctx.enter_context(tc.tile_pool(name="sx", bufs=2))
        pp = ctx.enter_context(tc.tile_pool(name="pp", bufs=2, space="PSUM"))
        sp = ctx.enter_context(tc.tile_pool(name="sp", bufs=6, space="PSUM"))

        # ---- SBUF tiles ----
        xt = sb.tile([128, 8 * XW], BF16, tag="xt")
        xn = sb.tile([128, 9 * 1024], BF16, tag="xn")
        qt = sb.tile([128, 8 * 1024], BF16, tag="qt")
        ptg = sb.tile([128, 8 * 256], BF16, tag="ptg")
        ysb = sb.tile([128, 8 * 1024], BF16, tag="ysb")
        draft = sb.tile([128, 8 * 1024], BF16, tag="draft")
        drs = sb.tile([128, 8 * 1024], BF16, tag="drs")
        h1 = sb.tile([128, 4 * 1024], BF16, tag="h1")
        statr = sb.tile([1, 1024], F32, tag="statr")
        statb = sb.tile([1, 2048], BF16, tag="statb")
        cb = sb.tile([128, 769], BF16, tag="cb")
        cf = sb.tile([128, 333], F32, tag="cf")
        aq = [sb.tile([128, 2048], BF16, tag=f"a{i}", name=f"aq{i}")
              for i in range(4)]
        wvq = [sb.tile([128, 2048], BF16, tag=f"wv{i}", name=f"wvq{i}")
               for i in range(4)]
        w1q = [sb.tile([128, 2048], BF16, tag=f"w1{i}", name=f"w1q{i}")
               for i in range(2)]
        w2h = [sb.tile([128, 2048], BF16, tag=f"w2{i}", name=f"w2h{i}")
               for i in range(2)]

        identb = cb[:, 0:128]
        ones_c = cb[:, 128:129]
        ones_r = cb[0:1, 129:257]
        nw1s = cb[0:1, 257:769]
        m0_sb = cf[:, 0:KWIN]
        mR_sb = cf[:, KWIN:2 * KWIN]
        b1c = cf[:, 320:324]
        b2c = cf[:, 324:332]
        eps_t = cf[0:1, 332:333]

        # ---- DMAs, in deadline order ----
        def dma_quarter(dst, src, i, ncols, nchunks):
            nc.sync.dma_start(
                dst[:, :].rearrange("p (c j) -> p c j", c=nchunks),
                src.rearrange("(c p) h -> p c h", p=128)
                [:, :, i * ncols:(i + 1) * ncols])

        xtv = xt[:, :].rearrange("p (c w) -> p c w", c=8)
        xTv = xT_d.rearrange("(c p) w -> p c w", p=128)
        dma_quarter(aq[0], a_d, 0, 256, 8)
        nc.sync.dma_start(xtv[:, :, WIN:WIN + 512], xTv[:, :, WIN:WIN + 512])
        nc.sync.dma_start(cb, cb_d)
        nc.sync.dma_start(cf, cf_d)
        for i in range(1, 4):
            dma_quarter(aq[i], a_d, i, 256, 8)
        nc.sync.dma_start(xtv[:, :, WIN + 512:XW], xTv[:, :, WIN + 512:XW])
        nc.sync.dma_start(xtv[:, :, 0:WIN], xTv[:, :, 0:WIN])
        nc.sync.dma_start(xn[:, :].rearrange("p (c h) -> p c h", c=9),
                          xn_d.rearrange("(c p) h -> p c h", p=128))
        for i in range(4):
            dma_quarter(wvq[i], wvo_d, i, 256, 8)
        for i in range(2):
            dma_quarter(w1q[i], w1_d, i, 256, 8)
        for i in range(2):
            dma_quarter(w2h[i], w2_d, i, 512, 4)

        # ---- PE warmup: junk transposes ramp the PE p-state while the
        # first weight/activation DMAs are in flight (no data deps).
        junk_sb = sx.tile([128, 128], BF16, tag="junk", bufs=1)
        nc.vector.memset(junk_sb[:, :], 0.0)
        junk_ps = sp.tile([128, 128], BF16, tag="sp", name="junk_ps")
        for _ in range(80):
            nc.tensor.transpose(junk_ps[:, 0:128], junk_sb[:, :],
                                junk_sb[:, :])

        # ---- Phase 1: qt = (x A)^T, layout [h-chunk][128, q] ----
        for qn in range(2):
            for i in range(4):
                for oc in (2 * i, 2 * i + 1):
                    ppt = pp.tile([128, 512], F32, tag="pp",
                                  name=f"q_{oc}_{qn}")
                    for kc in range(8):
                        nc.tensor.matmul(
                            ppt[:, :],
                            aq[i][:, kc * 256 + (oc % 2) * 128:
                                  kc * 256 + (oc % 2) * 128 + 128],
                            xt[:, kc * XW + WIN + qn * 512:
                               kc * XW + WIN + (qn + 1) * 512],
                            start=(kc == 0), stop=(kc == 7))
                    nc.any.tensor_copy(
                        qt[:, oc * 1024 + qn * 512:oc * 1024 + (qn + 1) * 512],
                        ppt[:, :])
        if DEBUG_TAPS:
            nc.sync.dma_start(taps["qt_d"], qt[:, :])

        # ---- Phase 2: local attention ----
        # Per 128-query block b, keys live at x^T cols [b*128, b*128+160);
        # the additive band mask (preloaded into PSUM) zeroes the rest.
        def sc_block(b):
            sct = sp.tile([128, KWIN], F32, tag="sp", name=f"sc{b}")
            for kc in range(8):
                nc.tensor.matmul(
                    sct[:, :],
                    qt[:, kc * 1024 + b * 128:kc * 1024 + (b + 1) * 128],
                    xt[:, kc * XW + b * 128:kc * XW + b * 128 + KWIN],
                    start=(kc == 0), stop=(kc == 7))
            return sct

        def softmax_block(b, sct):
            msl = m0_sb if b == 0 else mR_sb
            tt = sx.tile([128, KWIN], F32, tag="tt", name=f"tt{b}")
            nc.vector.tensor_add(tt[:, :], sct[:, :], msl)
            pexp = sx.tile([128, KWIN], F32, tag="pe", name=f"pe{b}")
            rsum = sx.tile([128, 1], F32, tag="rs", name=f"rs{b}")
            nc.scalar.activation(pexp[:, :], tt[:, :], AF.Exp,
                                 accum_out=rsum[:, 0:1])
            rcp = sx.tile([128, 1], F32, tag="rc", name=f"rc{b}")
            nc.vector.reciprocal(rcp[:, :], rsum[:, :])
            pn = sx.tile([128, KWIN], BF16, tag="pn", name=f"pn{b}")
            nc.scalar.activation(pn[:, :], pexp[:, :], AF.Copy,
                                 scale=rcp[:, 0:1])
            return pn

        def pt_block(b, pn):
            ptA = sp.tile([128, 128], BF16, tag="sp", name=f"ptA{b}")
            nc.tensor.transpose(ptA[:, 0:128], pn[:, 0:128], identb)
            nc.any.tensor_copy(ptg[:, b * 256:b * 256 + 128], ptA[:, 0:128])
            ptB = sp.tile([128, 128], BF16, tag="sp", name=f"ptB{b}")
            nc.tensor.transpose(ptB[0:32, 0:128], pn[:, 128:KWIN], identb)
            nc.any.tensor_copy(ptg[0:32, b * 256 + 128:b * 256 + 256],
                               ptB[0:32, 0:128])

        def y_group(oc, half):
            ypt = pp.tile([128, 512], F32, tag="pp", name=f"y{oc}_{half}")
            for j, bb in enumerate(range(half * 4, half * 4 + 4)):
                nc.tensor.matmul(
                    ypt[:, j * 128:(j + 1) * 128],
                    xn[:, bb * 1024 + oc * 128:bb * 1024 + oc * 128 + 128],
                    ptg[:, bb * 256:bb * 256 + 128],
                    start=(j == 0), stop=False)
            for j, bb in enumerate(range(half * 4, half * 4 + 4)):
                nc.tensor.matmul(
                    ypt[:, j * 128:(j + 1) * 128],
                    xn[0:32, (bb + 1) * 1024 + oc * 128:
                       (bb + 1) * 1024 + oc * 128 + 128],
                    ptg[0:32, bb * 256 + 128:bb * 256 + 256],
                    start=False, stop=(j == 3))
            base = oc * 1024 + half * 512
            nc.any.tensor_copy(ysb[:, base:base + 256], ypt[:, 0:256])
            nc.any.tensor_copy(ysb[:, base + 256:base + 512], ypt[:, 256:512])

        scts = {b: sc_block(b) for b in range(4)}
        for b in range(NB):
            pn = softmax_block(b, scts[b])
            if DEBUG_TAPS and b == 0:
                nc.sync.dma_start(taps["pn0_d"], pn[:, :])
            if b >= 4:
                y_group(2 * (b - 4), 0)
                y_group(2 * (b - 4) + 1, 0)
            pt_block(b, pn)
            if b + 4 < NB:
                scts[b + 4] = sc_block(b + 4)
        for oc in range(8):
            y_group(oc, 1)

        if DEBUG_TAPS:
            nc.sync.dma_start(taps["ptg_d"], ptg[:, :])
            nc.sync.dma_start(taps["ysb_d"], ysb[:, :])

        # ---- Phase 3: draftT = Wvo y + x^T; LN stats; drs = draft*rstd ----
        # qn-outer; stats matmuls trail their Wvo group by 2+ groups so the
        # PE never waits on the DVE add / Act square chain, and the qn=0
        # stats chain + rstd broadcast hide under the qn=1 Wvo matmuls.
        s_tiles = {}
        dsls = {}
        sqs = {}

        def wvo_group(qn, i, oc):
            ppt = pp.tile([128, 512], F32, tag="pp", name=f"wv_{oc}_{qn}")
            for kc in range(8):
                nc.tensor.matmul(
                    ppt[:, :],
                    wvq[i][:, kc * 256 + (oc % 2) * 128:
                           kc * 256 + (oc % 2) * 128 + 128],
                    ysb[:, kc * 1024 + qn * 512:kc * 1024 + (qn + 1) * 512],
                    start=(kc == 0), stop=(kc == 7))
            dsl = draft[:, oc * 1024 + qn * 512:oc * 1024 + (qn + 1) * 512]
            nc.vector.tensor_add(
                dsl, ppt[:, :],
                xt[:, oc * XW + WIN + qn * 512:oc * XW + WIN + (qn + 1) * 512])
            sq = sx.tile([128, 512], BF16, tag="sq", name=f"sq_{oc}_{qn}",
                         bufs=4)
            nc.scalar.square(sq[:, :], dsl)
            dsls[(qn, oc)] = dsl
            sqs[(qn, oc)] = sq

        def stat_mms(qn, oc):
            if oc == 0:
                s_tiles[qn] = (
                    sp.tile([1, 512], F32, tag="sp", name=f"s1_{qn}"),
                    sp.tile([1, 512], F32, tag="sp", name=f"s2_{qn}"))
            s1, s2 = s_tiles[qn]
            nc.tensor.matmul(s1[:, :], ones_c, dsls[(qn, oc)],
                             start=(oc == 0), stop=(oc == 7))
            nc.tensor.matmul(s2[:, :], ones_c, sqs[(qn, oc)][:, :],
                             start=(oc == 0), stop=(oc == 7))

        def stats_chain(qn):
            s1, s2 = s_tiles[qn]
            nc.vector.tensor_scalar_mul(s1[:, :], s1[:, :], 1.0 / H)
            mu2 = statr[0:1, qn * 512:(qn + 1) * 512]
            nc.scalar.square(mu2, s1[:, :])
            nc.vector.tensor_scalar_mul(s2[:, :], s2[:, :], 1.0 / H)
            nc.vector.tensor_sub(s2[:, :], s2[:, :], mu2)
            nc.scalar.activation(s2[:, :], s2[:, :], AF.Sqrt, bias=eps_t)
            rstd = statr[0:1, qn * 512:(qn + 1) * 512]
            nc.vector.reciprocal(rstd, s2[:, :])
            with nc.allow_low_precision(reason="bf16 operand copies of stats"):
                nc.vector.tensor_mul(
                    statb[0:1, 1024 + qn * 512:1024 + (qn + 1) * 512],
                    s1[:, :], rstd)
                nc.vector.tensor_copy(statb[0:1, qn * 512:(qn + 1) * 512],
                                      rstd)

        def rb_broadcast(qn):
            rb = pp.tile([128, 512], F32, tag="pp", name=f"rb{qn}")
            nc.tensor.matmul(rb[:, :], ones_r,
                             statb[0:1, qn * 512:(qn + 1) * 512],
                             start=True, stop=True)
            # rb rows are exact copies of bf16 rstd values -> bf16 is lossless
            rbs = sx.tile([128, 512], BF16, tag="rb", name=f"rbs{qn}")
            nc.any.tensor_copy(rbs[:, :], rb[:, :])
            return rbs

        def drs_muls(qn, rbs):
            # spread across DVE + Pool (Act has no tensor-tensor path)
            for oc in range(8):
                sl = slice(oc * 1024 + qn * 512, oc * 1024 + qn * 512 + 512)
                eng = nc.gpsimd if oc % 3 == 2 else nc.vector
                eng.tensor_mul(drs[:, sl], draft[:, sl], rbs[:, :])

        for i in range(4):
            for oc in (2 * i, 2 * i + 1):
                wvo_group(0, i, oc)
                if oc >= 2:
                    stat_mms(0, oc - 2)

        def mlp1_group(i, mc, qn):
            ppt = pp.tile([128, 512], F32, tag="pp", name=f"m1_{mc}_{qn}")
            for kc in range(8):
                nc.tensor.matmul(
                    ppt[:, :],
                    w1q[i][:, kc * 256 + (mc % 2) * 128:
                           kc * 256 + (mc % 2) * 128 + 128],
                    drs[:, kc * 1024 + qn * 512:kc * 1024 + (qn + 1) * 512],
                    start=(kc == 0), stop=False)
            nc.tensor.matmul(
                ppt[:, :],
                nw1s[0:1, mc * 128:(mc + 1) * 128],
                statb[0:1, 1024 + qn * 512:1024 + (qn + 1) * 512],
                start=False, stop=True)
            nc.scalar.activation(
                h1[:, mc * 1024 + qn * 512:mc * 1024 + (qn + 1) * 512],
                ppt[:, :], AF.Gelu, bias=b1c[:, mc:mc + 1], scale=1.0)

        # qn=1 Wvo groups, with the qn=0 stats tail / broadcast interleaved
        wvo_group(1, 0, 0)
        stat_mms(0, 6)
        wvo_group(1, 0, 1)
        stat_mms(0, 7)
        stats_chain(0)
        wvo_group(1, 1, 2)
        rbs0 = rb_broadcast(0)
        drs_muls(0, rbs0)
        wvo_group(1, 1, 3)
        stat_mms(1, 0)
        wvo_group(1, 2, 4)
        stat_mms(1, 1)
        wvo_group(1, 2, 5)
        stat_mms(1, 2)
        wvo_group(1, 3, 6)
        stat_mms(1, 3)
        wvo_group(1, 3, 7)
        stat_mms(1, 4)

        if DEBUG_TAPS:
            nc.sync.dma_start(taps["draft_d"], draft[:, :])

        # MLP1 qn=0 groups, with the qn=1 stats tail / broadcast interleaved
        mlp1_group(0, 0, 0)
        stat_mms(1, 5)
        stat_mms(1, 6)
        stat_mms(1, 7)
        stats_chain(1)
        mlp1_group(0, 1, 0)
        rbs1 = rb_broadcast(1)
        drs_muls(1, rbs1)
        mlp1_group(1, 2, 0)
        mlp1_group(1, 3, 0)
        for i in range(2):
            for mc in (2 * i, 2 * i + 1):
                mlp1_group(i, mc, 1)

        if DEBUG_TAPS:
            nc.sync.dma_start(taps["stat_d"], statr[:, :])
            nc.sync.dma_start(taps["statb_d"], statb[:, :])
            nc.sync.dma_start(taps["drs_d"], drs[:, :])
            nc.sync.dma_start(taps["h1_d"], h1[:, :])

        # ---- Phase 5: outT = W2 h1 + b2 + draftT ----
        # The +b2+draft adds are DVE-bound; every 3rd goes Act(PSUM copy) ->
        # Pool(SBUF stt) to keep the DVE off the critical path.
        for i in range(2):
            for oc in range(4 * i, 4 * i + 4):
                ot = sx.tile([128, 1024], BF16, tag="ot", bufs=3)
                for qn in range(2):
                    ppt = pp.tile([128, 512], F32, tag="pp",
                                  name=f"m2_{oc}_{qn}")
                    for mc in range(4):
                        nc.tensor.matmul(
                            ppt[:, :],
                            w2h[i][:, mc * 512 + (oc % 4) * 128:
                                   mc * 512 + (oc % 4) * 128 + 128],
                            h1[:, mc * 1024 + qn * 512:
                               mc * 1024 + (qn + 1) * 512],
                            start=(mc == 0), stop=(mc == 3))
                    dsl = draft[:, oc * 1024 + qn * 512:
                                oc * 1024 + (qn + 1) * 512]
                    if (oc * 2 + qn) % 3 == 2:
                        ppc = sx.tile([128, 512], BF16, tag="ppc", bufs=2,
                                      name=f"ppc_{oc}_{qn}")
                        nc.scalar.activation(ppc[:, :], ppt[:, :],
                                             AF.Identity,
                                             bias=b2c[:, oc:oc + 1])
                        nc.gpsimd.tensor_add(
                            ot[:, qn * 512:(qn + 1) * 512], ppc[:, :], dsl)
                    else:
                        nc.vector.scalar_tensor_tensor(
                            ot[:, qn * 512:(qn + 1) * 512], ppt[:, :],
                            b2c[:, oc:oc + 1], dsl,
                            op0=OP.add, op1=OP.add)
                nc.sync.dma_start(outT[oc * 128:(oc + 1) * 128, :], ot[:, :])

    nc.compile()
    return nc


def _get_nc():
    if "nc" not in _CACHE:
        _CACHE["nc"] = _build()
    return _CACHE["nc"]


def _masks():
    kk = np.arange(KWIN)[None, :]
    p = np.arange(128)[:, None]
    band = (kk - p >= 1) & (kk - p <= WIN)
    mR = np.where(band, 0.0, -1e30).astype(np.float32)
    m_first = np.where(band & (kk >= WIN), 0.0, -1e30).astype(np.float32)
    return m_first, mR


def kernel(hidden_states, Wq, Wk, Wv, Wo, ln_w, ln_b, W1, b1, W2, b2):
    bf16 = ml_dtypes.bfloat16
    hs = np.ascontiguousarray(np.asarray(hidden_states, np.float32))
    Wq, Wk, Wv, Wo = (np.asarray(a, np.float32) for a in (Wq, Wk, Wv, Wo))
    ln_w, ln_b = np.asarray(ln_w, np.float32), np.asarray(ln_b, np.float32)
    W1, b1 = np.asarray(W1, np.float32), np.asarray(b1, np.float32)
    W2, b2 = np.asarray(W2, np.float32), np.asarray(b2, np.float32)

    nc = _get_nc()
    m_first, mR = _masks()
    A = (Wq.T @ Wk) / np.sqrt(H)
    Wvo = Wo @ Wv
    w1T = W1.T * ln_w[:, None]
    cbm = np.zeros((128, 769), np.float32)
    cbm[:, 0:128] = np.eye(128, dtype=np.float32)
    cbm[:, 128] = 1.0
    cbm[0, 129:257] = 1.0
    cbm[0, 257:769] = -w1T.sum(0)

    def cf_pack(m0):
        cfm = np.zeros((128, 333), np.float32)
        cfm[:, 0:KWIN] = m0
        cfm[:, KWIN:2 * KWIN] = mR
        cfm[:, 320:324] = (b1 + W1 @ ln_b).reshape(4, 128).T
        cfm[:, 324:332] = b2.reshape(8, 128).T
        cfm[0, 332] = 1e-5
        return cfm
    cf_first, cf_rest = cf_pack(m_first), cf_pack(mR)
    shared = {
        "cb": cbm.astype(bf16),
        "a": np.ascontiguousarray(A).astype(bf16),
        "wvo": np.ascontiguousarray(Wvo.T).astype(bf16),
        "w1": np.ascontiguousarray(w1T).astype(bf16),
        "w2": np.ascontiguousarray(W2.T).astype(bf16),
    }
    in_maps = []
    for c in range(N_CORES):
        b, ch = divmod(c, 4)
        rows = hs[b, ch * SL:(ch + 1) * SL]
        halo = (np.zeros((WIN, H), np.float32) if ch == 0
                else hs[b, ch * SL - WIN:ch * SL])
        xfull = np.concatenate([halo, rows], 0)          # [1056, H]
        xn = np.zeros((9 * 128, H), np.float32)
        xn[0:XW] = xfull
        m = dict(shared)
        m["xT"] = np.ascontiguousarray(xfull.T).astype(bf16)
        m["xn"] = xn.astype(bf16)
        m["cf"] = cf_first if ch == 0 else cf_rest
        in_maps.append(m)

    res = run_bass_kernel_spmd(nc, in_maps, list(range(N_CORES)))
    _CACHE["res"] = res
    out = np.empty((B, S, H), np.float32)
    for c in range(N_CORES):
        b, ch = divmod(c, 4)
        out[b, ch * SL:(ch + 1) * SL] = \
            res.results[c]["outT"].T.astype(np.float32)
    return out


# revision 24
# speedup vs baseline: 1.6170x; 1.0014x over previous
"""LocalAttentionDraftLayer TRN2 Bass kernel (v2, bf16 + fused weights).

Sharding: sequence-parallel over B*S across 8 cores (each core owns a
contiguous 1024-token chunk of one batch row plus a 32-token halo of
preceding tokens, zero-padded at sequence start). Attention is strictly
local (window 32, causal), so the halo is materialized host-side and no
collectives are needed.

Key algebraic restructurings vs the straightforward layer (all host-side
weight folds, validated to rel err 4.6e-3 in numpy with bf16 operands):

  scores = (x Wq^T)(x Wk^T)^T / sqrt(H) = (x A) x^T  with A = Wq^T Wk / sqrt(H)
    -> the K projection disappears; keys are just x^T (already resident).
  attn_out = P (x Wv^T) Wo^T = (P x) Wvo^T          with Wvo = Wo Wv
    -> the V projection disappears; PV runs directly on x, and the old
       Wo-sized GEMM becomes the only post-attention projection.

Everything on-chip lives in "transposed land" ([feature, token]); all
matmul operands are bf16 (cost model: 1 cycle/row at any N, PSUM f32),
roughly 247K PE cycles/core vs 411K for the fp32r baseline.

Per core:
  qt[h,q]  = A^T x^T                      (q~ = xA, includes 1/sqrt(H))
  per 128-query block b (keys = 160-wide slice of x^T starting at b*128):
    sc = mask_preload + q~ . x            (PSUM-preloaded additive mask,
                                           accumulate with start=False)
    softmax without max-subtraction (scores are O(5); exp is safe):
    pexp = exp(sc) with accum -> rsum; rcp = 1/rsum (DVE);
    pn = pexp * rcp -> bf16 (Act copy-with-scale)
    P^T via two PE transposes (128 + 32 key rows) -> ptg (SBUF)
  y[h,q]   = x_nat^T P^T  per (h-chunk, 512-query half) PSUM group
  attnT    = Wvo y; draftT = attnT + x^T; LN stats via ones-matmul;
  rstd broadcast via K=1 matmul; mean correction as rank-1 matmul folded
  into MLP1 (ln_w folded into W1, ln_b folded into the gelu bias).
  h1 = gelu(W1w drs + nw1s*(mu*rstd) + b1c); outT = W2 h1 + b2 + draftT
Host transposes outT back and stitches the 8 chunks.
"""

import sys

sys.path.insert(0, "/opt/trn_rl_repo")

from contextlib import ExitStack

import numpy as np
import ml_dtypes

import concourse.bacc as bacc
import concourse.tile as tile
from concourse import mybir
from concourse.bass_utils import run_bass_kernel_spmd

B, S, H = 2, 4096, 1024
WIN = 32
N_CORES = 8
SL = S // 4            # 1024 tokens per core
XW = SL + WIN          # 1056 = halo + chunk
NB = SL // 128         # 8 query blocks
KWIN = 160             # keys per query block (128-aligned window cover)

F32 = mybir.dt.float32
BF16 = mybir.dt.bfloat16
OP = mybir.AluOpType
AF = mybir.ActivationFunctionType

_CACHE = {}
DEBUG_TAPS = False


def _build():
    nc = bacc.Bacc("TRN2", target_bir_lowering=False, debug=False,
                   num_devices=N_CORES)

    def din(name, shape, dt=BF16):
        return nc.dram_tensor(name, shape, dt, kind="ExternalInput").ap()

    xT_d = din("xT", [H, XW])
    xn_d = din("xn", [9 * 128, H])
    a_d = din("a", [H, H])
    wvo_d = din("wvo", [H, H])
    w1_d = din("w1", [H, 512])
    w2_d = din("w2", [512, H])
    cb_d = din("cb", [128, 769])          # ident|ones_c|ones_r(row0)|nw1s(row0)
    cf_d = din("cf", [128, 333], F32)     # m0|mR|b1c|b2c|eps
    outT = nc.dram_tensor("outT", [H, SL], BF16, kind="ExternalOutput").ap()
    taps = {}
    if DEBUG_TAPS:
        for nm, sh, dt in [("qt_d", [128, 8192], BF16),
                           ("pn0_d", [128, 160], BF16),
                           ("ptg_d", [128, 2048], BF16),
                           ("ysb_d", [128, 8192], BF16),
                           ("draft_d", [128, 8192], BF16),
                           ("drs_d", [128, 8192], BF16),
                           ("h1_d", [128, 4096], BF16),
                           ("stat_d", [1, 1024], F32),
                           ("statb_d", [1, 2048], BF16)]:
            taps[nm] = nc.dram_tensor(nm, sh, dt, kind="ExternalOutput").ap()

    with tile.TileContext(nc) as tc, ExitStack() as ctx:
        sb = ctx.enter_context(tc.tile_pool(name="sb", bufs=1))
        sx = ctx.enter_context(tc.tile_pool(name="sx", bufs=2))
        pp = ctx.enter_context(tc.tile_pool(name="pp", bufs=2, space="PSUM"))
        sp = ctx.enter_context(tc.tile_pool(name="sp", bufs=4, space="PSUM"))
        pt = ctx.enter_context(tc.tile_pool(name="pt", bufs=2, space="PSUM"))

        # ---- SBUF tiles ----
        xt = sb.tile([128, 8 * XW], BF16, tag="xt")
        xn = sb.tile([128, 9 * 1024], BF16, tag="xn")
        qt = sb.tile([128, 8 * 1024], BF16, tag="qt")
        ptg = sb.tile([128, 8 * 256], BF16, tag="ptg")
        ysb = sb.tile([128, 8 * 1024], BF16, tag="ysb")
        draft = sb.tile([128, 8 * 1024], BF16, tag="draft")
        drs = sb.tile([128, 8 * 1024], BF16, tag="drs")
        h1 = sb.tile([128, 4 * 1024], BF16, tag="h1")
        statr = sb.tile([1, 1024], F32, tag="statr")
        statb = sb.tile([1, 2048], BF16, tag="statb")
        cb = sb.tile([128, 769], BF16, tag="cb")
        cf = sb.tile([128, 333], F32, tag="cf")
        aq = [sb.tile([128, 2048], BF16, tag=f"a{i}", name=f"aq{i}")
              for i in range(4)]
        wvq = [sb.tile([128, 2048], BF16, tag=f"wv{i}", name=f"wvq{i}")
               for i in range(4)]
        w1q = [sb.tile([128, 2048], BF16, tag=f"w1{i}", name=f"w1q{i}")
               for i in range(2)]
        w2h = [sb.tile([128, 2048], BF16, tag=f"w2{i}", name=f"w2h{i}")
               for i in range(2)]

        identb = cb[:, 0:128]
        ones_c = cb[:, 128:129]
        ones_r = cb[0:1, 129:257]
        nw1s = cb[0:1, 257:769]
        m0_sb = cf[:, 0:KWIN]
        mR_sb = cf[:, KWIN:2 * KWIN]
        b1c = cf[:, 320:324]
        b2c = cf[:, 324:332]
        eps_t = cf[0:1, 332:333]

        # ---- DMAs, in deadline order ----
        def dma_quarter(dst, src, i, ncols, nchunks):
            nc.sync.dma_start(
                dst[:, :].rearrange("p (c j) -> p c j", c=nchunks),
                src.rearrange("(c p) h -> p c h", p=128)
                [:, :, i * ncols:(i + 1) * ncols])

        xtv = xt[:, :].rearrange("p (c w) -> p c w", c=8)
        xTv = xT_d.rearrange("(c p) w -> p c w", p=128)
        dma_quarter(aq[0], a_d, 0, 256, 8)
        nc.sync.dma_start(xtv[:, :, WIN:WIN + 512], xTv[:, :, WIN:WIN + 512])
        nc.sync.dma_start(cb, cb_d)
        nc.sync.dma_start(cf, cf_d)
        for i in range(1, 4):
            dma_quarter(aq[i], a_d, i, 256, 8)
        nc.sync.dma_start(xtv[:, :, WIN + 512:XW], xTv[:, :, WIN + 512:XW])
        nc.sync.dma_start(xtv[:, :, 0:WIN], xTv[:, :, 0:WIN])
        nc.sync.dma_start(xn[:, :].rearrange("p (c h) -> p c h", c=9),
                          xn_d.rearrange("(c p) h -> p c h", p=128))
        for i in range(4):
            dma_quarter(wvq[i], wvo_d, i, 256, 8)
        for i in range(2):
            dma_quarter(w1q[i], w1_d, i, 256, 8)
        for i in range(2):
            dma_quarter(w2h[i], w2_d, i, 512, 4)

        # ---- PE warmup: junk transposes ramp the PE p-state while the
        # first weight/activation DMAs are in flight (no data deps).
        junk_sb = sx.tile([128, 128], BF16, tag="junk", bufs=1)
        nc.vector.memset(junk_sb[:, :], 0.0)
        junk_f = sx.tile([1, 1], F32, tag="junkf", bufs=1)
        # preload the Exp act-table set while the PE warms up
        nc.scalar.activation(junk_f[:, :], junk_sb[0:1, 0:1], AF.Exp)
        junk_ps = sp.tile([128, 128], BF16, tag="sp", name="junk_ps")
        for _ in range(80):
            nc.tensor.transpose(junk_ps[:, 0:128], junk_sb[:, :],
                                junk_sb[:, :])

        # ---- Phase 1: qt = (x A)^T, layout [h-chunk][128, q] ----
        for qn in range(2):
            for i in range(4):
                for oc in (2 * i, 2 * i + 1):
                    ppt = pp.tile([128, 512], F32, tag="pp",
                                  name=f"q_{oc}_{qn}")
                    for kc in range(8):
                        nc.tensor.matmul(
                            ppt[:, :],
                            aq[i][:, kc * 256 + (oc % 2) * 128:
                                  kc * 256 + (oc % 2) * 128 + 128],
                            xt[:, kc * XW + WIN + qn * 512:
                               kc * XW + WIN + (qn + 1) * 512],
                            start=(kc == 0), stop=(kc == 7))
                    nc.any.tensor_copy(
                        qt[:, oc * 1024 + qn * 512:oc * 1024 + (qn + 1) * 512],
                        ppt[:, :])
        if DEBUG_TAPS:
            nc.sync.dma_start(taps["qt_d"], qt[:, :])

        # ---- Phase 2: local attention ----
        # Per 128-query block b, keys live at x^T cols [b*128, b*128+160);
        # the additive band mask (preloaded into PSUM) zeroes the rest.
        def sc_block(b):
            sct = sp.tile([128, KWIN], F32, tag="sp", name=f"sc{b}")
            for kc in range(8):
                nc.tensor.matmul(
                    sct[:, :],
                    qt[:, kc * 1024 + b * 128:kc * 1024 + (b + 1) * 128],
                    xt[:, kc * XW + b * 128:kc * XW + b * 128 + KWIN],
                    start=(kc == 0), stop=(kc == 7))
            return sct

        def softmax_block(b, sct):
            msl = m0_sb if b == 0 else mR_sb
            tt = sx.tile([128, KWIN], F32, tag="tt", name=f"tt{b}")
            nc.vector.tensor_add(tt[:, :], sct[:, :], msl)
            pexp = sx.tile([128, KWIN], F32, tag="pe", name=f"pe{b}")
            rsum = sx.tile([128, 1], F32, tag="rs", name=f"rs{b}")
            nc.scalar.activation(pexp[:, :], tt[:, :], AF.Exp,
                                 accum_out=rsum[:, 0:1])
            rcp = sx.tile([128, 1], F32, tag="rc", name=f"rc{b}")
            nc.vector.reciprocal(rcp[:, :], rsum[:, :])
            pn = sx.tile([128, KWIN], BF16, tag="pn", name=f"pn{b}")
            nc.gpsimd.tensor_scalar_mul(pn[:, :], pexp[:, :], rcp[:, 0:1])
            return pn

        def pt_block(b, pn):
            ptA = pt.tile([128, 128], BF16, tag="pt", name=f"ptA{b}")
            nc.tensor.transpose(ptA[:, 0:128], pn[:, 0:128], identb)
            nc.any.tensor_copy(ptg[:, b * 256:b * 256 + 128], ptA[:, 0:128])
            ptB = pt.tile([128, 128], BF16, tag="pt", name=f"ptB{b}")
            nc.tensor.transpose(ptB[0:32, 0:128], pn[:, 128:KWIN], identb)
            nc.any.tensor_copy(ptg[0:32, b * 256 + 128:b * 256 + 256],
                               ptB[0:32, 0:128])

        def y_group(oc, half):
            ypt = pp.tile([128, 512], F32, tag="pp", name=f"y{oc}_{half}")
            for j, bb in enumerate(range(half * 4, half * 4 + 4)):
                nc.tensor.matmul(
                    ypt[:, j * 128:(j + 1) * 128],
                    xn[:, bb * 1024 + oc * 128:bb * 1024 + oc * 128 + 128],
                    ptg[:, bb * 256:bb * 256 + 128],
                    start=(j == 0), stop=False)
            for j, bb in enumerate(range(half * 4, half * 4 + 4)):
                nc.tensor.matmul(
                    ypt[:, j * 128:(j + 1) * 128],
                    xn[0:32, (bb + 1) * 1024 + oc * 128:
                       (bb + 1) * 1024 + oc * 128 + 128],
                    ptg[0:32, bb * 256 + 128:bb * 256 + 256],
                    start=False, stop=(j == 3))
            base = oc * 1024 + half * 512
            nc.any.tensor_copy(ysb[:, base:base + 256], ypt[:, 0:256])
            nc.any.tensor_copy(ysb[:, base + 256:base + 512], ypt[:, 256:512])

        scts = {b: sc_block(b) for b in range(4)}
        for b in range(NB):
            pn = softmax_block(b, scts[b])
            if DEBUG_TAPS and b == 0:
                nc.sync.dma_start(taps["pn0_d"], pn[:, :])
            if b >= 4:
                y_group(2 * (b - 4), 0)
                y_group(2 * (b - 4) + 1, 0)
            if b + 4 < NB:
                scts[b + 4] = sc_block(b + 4)
            pt_block(b, pn)
        for oc in range(8):
            y_group(oc, 1)

        if DEBUG_TAPS:
            nc.sync.dma_start(taps["ptg_d"], ptg[:, :])
            nc.sync.dma_start(taps["ysb_d"], ysb[:, :])

        # ---- Phase 3: draftT = Wvo y + x^T; LN stats; drs = draft*rstd ----
        # qn-outer; stats matmuls trail their Wvo group by 2+ groups so the
        # PE never waits on the DVE add / Act square chain, and the qn=0
        # stats chain + rstd broadcast hide under the qn=1 Wvo matmuls.
        s_tiles = {}
        dsls = {}
        sqs = {}

        def wvo_group(qn, i, oc):
            ppt = pp.tile([128, 512], F32, tag="pp", name=f"wv_{oc}_{qn}")
            for kc in range(8):
                nc.tensor.matmul(
                    ppt[:, :],
                    wvq[i][:, kc * 256 + (oc % 2) * 128:
                           kc * 256 + (oc % 2) * 128 + 128],
                    ysb[:, kc * 1024 + qn * 512:kc * 1024 + (qn + 1) * 512],
                    start=(kc == 0), stop=(kc == 7))
            dsl = draft[:, oc * 1024 + qn * 512:oc * 1024 + (qn + 1) * 512]
            nc.vector.tensor_add(
                dsl, ppt[:, :],
                xt[:, oc * XW + WIN + qn * 512:oc * XW + WIN + (qn + 1) * 512])
            sq = sx.tile([128, 512], BF16, tag="sq", name=f"sq_{oc}_{qn}",
                         bufs=4)
            nc.scalar.square(sq[:, :], dsl)
            dsls[(qn, oc)] = dsl
            sqs[(qn, oc)] = sq

        def stat_mms(qn, oc):
            if oc == 0:
                s_tiles[qn] = (
                    sp.tile([1, 512], F32, tag="sp", name=f"s1_{qn}"),
                    sp.tile([1, 512], F32, tag="sp", name=f"s2_{qn}"))
            s1, s2 = s_tiles[qn]
            nc.tensor.matmul(s1[:, :], ones_c, dsls[(qn, oc)],
                             start=(oc == 0), stop=(oc == 7))
            nc.tensor.matmul(s2[:, :], ones_c, sqs[(qn, oc)][:, :],
                             start=(oc == 0), stop=(oc == 7))

        def stats_chain(qn):
            s1, s2 = s_tiles[qn]
            nc.vector.tensor_scalar_mul(s1[:, :], s1[:, :], 1.0 / H)
            mu2 = statr[0:1, qn * 512:(qn + 1) * 512]
            nc.scalar.square(mu2, s1[:, :])
            nc.vector.tensor_scalar_mul(s2[:, :], s2[:, :], 1.0 / H)
            nc.vector.tensor_sub(s2[:, :], s2[:, :], mu2)
            nc.scalar.activation(s2[:, :], s2[:, :], AF.Sqrt, bias=eps_t)
            rstd = statr[0:1, qn * 512:(qn + 1) * 512]
            nc.vector.reciprocal(rstd, s2[:, :])
            with nc.allow_low_precision(reason="bf16 operand copies of stats"):
                nc.vector.tensor_mul(
                    statb[0:1, 1024 + qn * 512:1024 + (qn + 1) * 512],
                    s1[:, :], rstd)
                nc.vector.tensor_copy(statb[0:1, qn * 512:(qn + 1) * 512],
                                      rstd)

        def rb_broadcast(qn):
            rb = pp.tile([128, 512], F32, tag="pp", name=f"rb{qn}")
            nc.tensor.matmul(rb[:, :], ones_r,
                             statb[0:1, qn * 512:(qn + 1) * 512],
                             start=True, stop=True)
            # rb rows are exact copies of bf16 rstd values -> bf16 is lossless
            rbs = sx.tile([128, 512], BF16, tag="rb", name=f"rbs{qn}")
            nc.any.tensor_copy(rbs[:, :], rb[:, :])
            return rbs

        def drs_muls(qn, rbs):
            # Pool-heavy: the DVE queue is busy with residual adds here
            for oc in range(8):
                sl = slice(oc * 1024 + qn * 512, oc * 1024 + qn * 512 + 512)
                eng = nc.vector if oc in (3, 7) else nc.gpsimd
                eng.tensor_mul(drs[:, sl], draft[:, sl], rbs[:, :])

        for i in range(4):
            for oc in (2 * i, 2 * i + 1):
                wvo_group(0, i, oc)
                if oc >= 2:
                    stat_mms(0, oc - 2)

        def mlp1_group(i, mc, qn):
            ppt = pp.tile([128, 512], F32, tag="pp", name=f"m1_{mc}_{qn}")
            for kc in range(8):
                nc.tensor.matmul(
                    ppt[:, :],
                    w1q[i][:, kc * 256 + (mc % 2) * 128:
                           kc * 256 + (mc % 2) * 128 + 128],
                    drs[:, kc * 1024 + qn * 512:kc * 1024 + (qn + 1) * 512],
                    start=(kc == 0), stop=False)
            nc.tensor.matmul(
                ppt[:, :],
                nw1s[0:1, mc * 128:(mc + 1) * 128],
                statb[0:1, 1024 + qn * 512:1024 + (qn + 1) * 512],
                start=False, stop=True)
            nc.scalar.activation(
                h1[:, mc * 1024 + qn * 512:mc * 1024 + (qn + 1) * 512],
                ppt[:, :], AF.Gelu, bias=b1c[:, mc:mc + 1], scale=1.0)

        # qn=1 Wvo groups, with the qn=0 stats tail / broadcast interleaved
        wvo_group(1, 0, 0)
        stat_mms(0, 6)
        wvo_group(1, 0, 1)
        stat_mms(0, 7)
        stats_chain(0)
        wvo_group(1, 1, 2)
        rbs0 = rb_broadcast(0)
        drs_muls(0, rbs0)
        wvo_group(1, 1, 3)
        stat_mms(1, 0)
        wvo_group(1, 2, 4)
        stat_mms(1, 1)
        wvo_group(1, 2, 5)
        stat_mms(1, 2)
        wvo_group(1, 3, 6)
        stat_mms(1, 3)
        stat_mms(1, 4)
        wvo_group(1, 3, 7)
        stat_mms(1, 5)
        stat_mms(1, 6)
        stat_mms(1, 7)
        # whole qn=1 chain ahead of any gelu: its Sqrt must hit the Act
        # queue before the Gelu act-table load (set switches cost 1.28us)
        stats_chain(1)

        if DEBUG_TAPS:
            nc.sync.dma_start(taps["draft_d"], draft[:, :])

        # MLP1 qn=0 groups, with the qn=1 rstd broadcast interleaved
        mlp1_group(0, 0, 0)
        rbs1 = rb_broadcast(1)
        drs_muls(1, rbs1)
        mlp1_group(0, 1, 0)
        mlp1_group(1, 2, 0)
        mlp1_group(1, 3, 0)
        for i in range(2):
            for mc in (2 * i, 2 * i + 1):
                mlp1_group(i, mc, 1)

        if DEBUG_TAPS:
            nc.sync.dma_start(taps["stat_d"], statr[:, :])
            nc.sync.dma_start(taps["statb_d"], statb[:, :])
            nc.sync.dma_start(taps["drs_d"], drs[:, :])
            nc.sync.dma_start(taps["h1_d"], h1[:, :])

        # ---- Phase 5: outT = W2 h1 + b2 + draftT ----
        # The +b2+draft adds are DVE-bound; every 3rd goes Act(PSUM copy) ->
        # Pool(SBUF stt) to keep the DVE off the critical path.
        for i in range(2):
            for oc in range(4 * i, 4 * i + 4):
                ot = sx.tile([128, 1024], BF16, tag="ot", bufs=3)
                for qn in range(2):
                    ppt = pp.tile([128, 512], F32, tag="pp",
                                  name=f"m2_{oc}_{qn}")
                    for mc in range(4):
                        nc.tensor.matmul(
                            ppt[:, :],
                            w2h[i][:, mc * 512 + (oc % 4) * 128:
                                   mc * 512 + (oc % 4) * 128 + 128],
                            h1[:, mc * 1024 + qn * 512:
                               mc * 1024 + (qn + 1) * 512],
                            start=(mc == 0), stop=(mc == 3))
                    dsl = draft[:, oc * 1024 + qn * 512:
                                oc * 1024 + (qn + 1) * 512]
                    if (oc * 2 + qn) % 3 == 2:
                        ppc = sx.tile([128, 512], BF16, tag="ppc", bufs=2,
                                      name=f"ppc_{oc}_{qn}")
                        nc.scalar.activation(ppc[:, :], ppt[:, :],
                                             AF.Identity,
                                             bias=b2c[:, oc:oc + 1])
                        nc.gpsimd.tensor_add(
                            ot[:, qn * 512:(qn + 1) * 512], ppc[:, :], dsl)
                    else:
                        nc.vector.scalar_tensor_tensor(
                            ot[:, qn * 512:(qn + 1) * 512], ppt[:, :],
                            b2c[:, oc:oc + 1], dsl,
                            op0=OP.add, op1=OP.add)
                    nc.sync.dma_start(
                        outT[oc * 128:(oc + 1) * 128,
                             qn * 512:(qn + 1) * 512],
                        ot[:, qn * 512:(qn + 1) * 512])

    nc.compile()
    return nc


def _get_nc():
    if "nc" not in _CACHE:
        _CACHE["nc"] = _build()
    return _CACHE["nc"]


def _masks():
    kk = np.arange(KWIN)[None, :]
    p = np.arange(128)[:, None]
    band = (kk - p >= 1) & (kk - p <= WIN)
    mR = np.where(band, 0.0, -1e30).astype(np.float32)
    m_first = np.where(band & (kk >= WIN), 0.0, -1e30).astype(np.float32)
    return m_first, mR


def kernel(hidden_states, Wq, Wk, Wv, Wo, ln_w, ln_b, W1, b1, W2, b2):
    bf16 = ml_dtypes.bfloat16
    hs = np.ascontiguousarray(np.asarray(hidden_states, np.float32))
    Wq, Wk, Wv, Wo = (np.asarray(a, np.float32) for a in (Wq, Wk, Wv, Wo))
    ln_w, ln_b = np.asarray(ln_w, np.float32), np.asarray(ln_b, np.float32)
    W1, b1 = np.asarray(W1, np.float32), np.asarray(b1, np.float32)
    W2, b2 = np.asarray(W2, np.float32), np.asarray(b2, np.float32)

    nc = _get_nc()
    m_first, mR = _masks()
    A = (Wq.T @ Wk) / np.sqrt(H)
    Wvo = Wo @ Wv
    w1T = W1.T * ln_w[:, None]
    cbm = np.zeros((128, 769), np.float32)
    cbm[:, 0:128] = np.eye(128, dtype=np.float32)
    cbm[:, 128] = 1.0
    cbm[0, 129:257] = 1.0
    cbm[0, 257:769] = -w1T.sum(0)

    def cf_pack(m0):
        cfm = np.zeros((128, 333), np.float32)
        cfm[:, 0:KWIN] = m0
        cfm[:, KWIN:2 * KWIN] = mR
        cfm[:, 320:324] = (b1 + W1 @ ln_b).reshape(4, 128).T
        cfm[:, 324:332] = b2.reshape(8, 128).T
        cfm[0, 332] = 1e-5
        return cfm
    cf_first, cf_rest = cf_pack(m_first), cf_pack(mR)
    shared = {
        "cb": cbm.astype(bf16),
        "a": np.ascontiguousarray(A).astype(bf16),
        "wvo": np.ascontiguousarray(Wvo.T).astype(bf16),
        "w1": np.ascontiguousarray(w1T).astype(bf16),
        "w2": np.ascontiguousarray(W2.T).astype(bf16),
    }
    in_maps = []
    for c in range(N_CORES):
        b, ch = divmod(c, 4)
        rows = hs[b, ch * SL:(ch + 1) * SL]
        halo = (np.zeros((WIN, H), np.float32) if ch == 0
                else hs[b, ch * SL - WIN:ch * SL])
        xfull = np.concatenate([halo, rows], 0)          # [1056, H]
        xn = np.zeros((9 * 128, H), np.float32)
        xn[0:XW] = xfull
        m = dict(shared)
        m["xT"] = np.ascontiguousarray(xfull.T).astype(bf16)
        m["xn"] = xn.astype(bf16)
        m["cf"] = cf_first if ch == 0 else cf_rest
        in_maps.append(m)

    res = run_bass_kernel_spmd(nc, in_maps, list(range(N_CORES)))
    _CACHE["res"] = res
    out = np.empty((B, S, H), np.float32)
    for c in range(N_CORES):
        b, ch = divmod(c, 4)
        out[b, ch * SL:(ch + 1) * SL] = \
            res.results[c]["outT"].T.astype(np.float32)
    return out
